# revision 66
# baseline (speedup 1.0000x reference)
"""Multi-head attention (B=4, S=2048, D=1024, H=16) on 8 Trainium2 cores.

Sharding: core c -> (batch b = c//2, head-half hh = c%2).  Each core computes
8 heads of one batch: QKV projections with column-sliced weights, attention,
and a partial output projection with row-sliced Wo.  Host sums the two
partial outputs per batch and adds the constant bias terms.

All matmul operands are bf16 (1 cyc/row streaming + FWL weight loads); host
pre-casts inputs/weights to bf16 so DMAs don't convert.  Everything is
computed in "transposed" orientation (features/keys on partitions):
  - Q^T, K^T: [512 feat, 2048 tok] via lhsT=W chunk, rhs=X^T chunk
  - V natural: [2048 tok, 512 feat] via lhsT=X^T chunk, rhs=Wv chunk
  - S^T[k,q]: lhsT=K^T[64,kblk], rhs=Q^T[64,qchunk] (two heads row-packed,
    concurrent in the PE via row groups)
  - P^T = exp(S^T) on ACT, PSUM->SBUF bf16 (no max-subtraction: |scores|<~6)
  - PV: out^T[dk,q] accum over kblk (two heads col-packed, concurrent); Z row
    sums via lhsT=ones alongside.  PV/Z for kblk i are emitted AFTER the
    scores matmul of kblk i+1 (software pipelining over the exp).
  - softmax normalization off the critical path: 1/Z via fast approx
    reciprocal on DVE, rounded to f32r, broadcast via rank-1 selector matmul,
    applied to x^T with a DVE mul
  - y[q, fout] natural: lhsT=x^T slice, rhs=Wo rows; per-128-row output
    blocks stream to HBM as soon as the last head block is normalized

The attention phase is ACT(exp)-bound (~1.15us per [128,1024] tile), so the
kernel minimizes the serial head before the first exp (V proj + K proj +
Q m=0 only) and interleaves the remaining Q projection chunks and the whole
output projection into the PE's slack inside the attention loop via a work
queue drained at kb boundaries.
"""
import numpy as np
import ml_dtypes

import concourse.tile as tile
from concourse import bacc, mybir
from concourse import bass_utils

F32 = mybir.dt.float32
F32R = mybir.dt.float32r
BF16 = mybir.dt.bfloat16
EXP = mybir.ActivationFunctionType.Exp

B, S, D = 4, 2048, 1024
H = 16
DK = 64
FEAT = 512          # features per core (8 heads)
N_CORES = 8

_PROGRAM = None


def _build_program():
    nc = bacc.Bacc("TRN2", target_bir_lowering=False, debug=False,
                   enable_asserts=True, num_devices=N_CORES)

    xq = nc.dram_tensor("xq_t", [D, S], BF16, kind="ExternalInput").ap()
    xk = nc.dram_tensor("xk_t", [D, S], BF16, kind="ExternalInput").ap()
    xv = nc.dram_tensor("xv_t", [D, S], BF16, kind="ExternalInput").ap()
    wq = nc.dram_tensor("wq", [D, FEAT], BF16, kind="ExternalInput").ap()
    wk = nc.dram_tensor("wk", [D, FEAT], BF16, kind="ExternalInput").ap()
    wv = nc.dram_tensor("wv", [D, FEAT], BF16, kind="ExternalInput").ap()
    wo = nc.dram_tensor("wo", [FEAT, D], BF16, kind="ExternalInput").ap()
    bq = nc.dram_tensor("bq", [FEAT, 1], F32, kind="ExternalInput").ap()
    bk = nc.dram_tensor("bk", [FEAT, 1], F32, kind="ExternalInput").ap()
    sel = nc.dram_tensor("sel", [2, 128], F32, kind="ExternalInput").ap()
    ones = nc.dram_tensor("ones", [128, 64], BF16, kind="ExternalInput").ap()
    y = nc.dram_tensor("y", [S, D], F32, kind="ExternalOutput").ap()

    with tile.TileContext(nc) as tc:
        with nc.allow_low_precision(reason="bf16/fp32r matmul operand tiles"):
            _emit(nc, tc, xq, xk, xv, wq, wk, wv, wo, bq, bk, sel, ones, y)
    nc.compile()
    return nc


def _emit(nc, tc, xq, xk, xv, wq, wk, wv, wo, bq, bk, sel, ones, y):
    from contextlib import ExitStack

    MM = nc.tensor.matmul

    with ExitStack() as ctx:
        ep = ctx.enter_context

        # ---------- persistent SBUF ----------
        qt_pool = ep(tc.tile_pool(name="qt", bufs=1))
        kt_pool = ep(tc.tile_pool(name="kt", bufs=1))
        v_pool = ep(tc.tile_pool(name="v", bufs=1))
        misc_pool = ep(tc.tile_pool(name="misc", bufs=1))
        xT_pool = ep(tc.tile_pool(name="xT", bufs=1))
        pt_pool = ep(tc.tile_pool(name="pt", bufs=3))
        rz_pool = ep(tc.tile_pool(name="rz", bufs=1))
        y_sb_pool = ep(tc.tile_pool(name="ysb", bufs=2))
        xk_pool = ep(tc.tile_pool(name="xk", bufs=1))
        wk_pool = ep(tc.tile_pool(name="wkp", bufs=1))
        xq_pool = ep(tc.tile_pool(name="xqp", bufs=1))
        wq_pool = ep(tc.tile_pool(name="wqp", bufs=1))

        qt = [qt_pool.tile([128, S], BF16, tag=f"qt{m}", name=f"qt{m}") for m in range(4)]
        kt = [kt_pool.tile([128, S], BF16, tag=f"kt{m}", name=f"kt{m}") for m in range(4)]
        v_sb = [v_pool.tile([128, FEAT], BF16, tag=f"v{k}", name=f"v{k}") for k in range(16)]
        xT = [xT_pool.tile([128, S], BF16, tag=f"xT{p}", name=f"xT{p}") for p in range(4)]

        bq_sb = misc_pool.tile([128, 4], F32, tag="bq")
        bk_sb = misc_pool.tile([128, 4], F32, tag="bk")
        ones_sb = misc_pool.tile([128, 64], BF16, tag="ones")
        selA_sb = misc_pool.tile([1, 128], F32R, tag="selA")
        selB_sb = misc_pool.tile([1, 128], F32R, tag="selB")

        xk_sb = [xk_pool.tile([128, S], BF16, tag=f"xk{c}", name=f"xk{c}") for c in range(8)]
        wk_sb = [wk_pool.tile([128, FEAT], BF16, tag=f"wk{c}", name=f"wk{c}") for c in range(8)]
        xq_sb = [xq_pool.tile([128, S], BF16, tag=f"xq{c}", name=f"xq{c}") for c in range(8)]
        wq_sb = [wq_pool.tile([128, FEAT], BF16, tag=f"wq{c}", name=f"wq{c}") for c in range(8)]

        # spread input DMAs across engine queues (one queue ~160GB/s):
        # V inputs split 3 ways so the V projection starts ASAP; K then Q
        # inputs follow on sync/scalar; weights trail on gpsimd/scalar.
        for m in range(4):
            nc.sync.dma_start(bq_sb[:, m:m + 1], bq[m * 128:(m + 1) * 128, 0:1])
            nc.sync.dma_start(bk_sb[:, m:m + 1], bk[m * 128:(m + 1) * 128, 0:1])
        nc.sync.dma_start(selA_sb[:], sel[0:1, :].bitcast(F32R))
        nc.sync.dma_start(selB_sb[:], sel[1:2, :].bitcast(F32R))

        # ---------- V projection (natural: [2048 tok, 512 feat]) ----------
        with tc.tile_pool(name="xv", bufs=1) as xv_pool, \
             tc.tile_pool(name="wvp", bufs=1) as wv_pool:
            xv_sb = [xv_pool.tile([128, S], BF16, tag=f"xv{c}", name=f"xv{c}")
                     for c in range(8)]
            wv_sb = [wv_pool.tile([128, FEAT], BF16, tag=f"wv{c}", name=f"wv{c}")
                     for c in range(8)]
            nc.gpsimd.dma_start(ones_sb[:], ones)
            for c in range(2):
                nc.gpsimd.dma_start(wv_sb[c][:], wv[c * 128:(c + 1) * 128, :])
                nc.gpsimd.dma_start(xv_sb[c][:], xv[c * 128:(c + 1) * 128, :])
            for c in range(2, 4):
                nc.scalar.dma_start(wv_sb[c][:], wv[c * 128:(c + 1) * 128, :])
                nc.scalar.dma_start(xv_sb[c][:], xv[c * 128:(c + 1) * 128, :])
            for c in range(4, 8):
                nc.sync.dma_start(wv_sb[c][:], wv[c * 128:(c + 1) * 128, :])
                nc.sync.dma_start(xv_sb[c][:], xv[c * 128:(c + 1) * 128, :])
            # K inputs on sync, Q inputs split sync/scalar, weights trail
            for c in range(8):
                nc.sync.dma_start(wk_sb[c][:], wk[c * 128:(c + 1) * 128, :])
                nc.sync.dma_start(xk_sb[c][:], xk[c * 128:(c + 1) * 128, :])
            for c in range(8):
                nc.gpsimd.dma_start(wq_sb[c][:], wq[c * 128:(c + 1) * 128, :])
            for c in range(4):
                nc.sync.dma_start(xq_sb[c][:], xq[c * 128:(c + 1) * 128, :])
            for c in range(4, 8):
                nc.scalar.dma_start(xq_sb[c][:], xq[c * 128:(c + 1) * 128, :])
            with tc.tile_pool(name="vps", bufs=1, space="PSUM") as vps_pool:
                for khalf in range(2):
                    vps = [vps_pool.tile([128, FEAT], F32, tag=f"vps{j}", name=f"vps{j}")
                           for j in range(8)]
                    for c in range(8):
                        for j in range(8):
                            kb = khalf * 8 + j
                            MM(vps[j][:],
                               xv_sb[c][:, kb * 128:(kb + 1) * 128],
                               wv_sb[c][:],
                               start=(c == 0), stop=(c == 7))
                    for j in range(8):
                        # split PSUM->SBUF copies across DVE and the (still
                        # idle) ACT engine so the K/Q bias-adds aren't stuck
                        # behind them on the vector queue
                        if khalf == 0:
                            nc.vector.tensor_copy(v_sb[khalf * 8 + j][:], vps[j][:])
                        else:
                            nc.scalar.activation(
                                v_sb[khalf * 8 + j][:], vps[j][:],
                                mybir.ActivationFunctionType.Copy)

        # wo loads into the SBUF range freed by xv (needed only from ~p2 on)
        wo_pool = ep(tc.tile_pool(name="wo", bufs=1))
        wo_sb = [wo_pool.tile([128, D], BF16, tag=f"wo{p}", name=f"wo{p}") for p in range(4)]
        for p in range(4):
            nc.scalar.dma_start(wo_sb[p][:], wo[p * 128:(p + 1) * 128, :])

        # ---------- attention (ACT-bound) with interleaved PE work ----------
        # Pre-attention head is only K m=0 n=0 and Q m=0 n=0; every other
        # projection chunk plus the whole output projection is drained from a
        # work queue inside the ACT-paced attention loop.
        with tc.tile_pool(name="st", bufs=2, space="PSUM") as st_pool, \
             tc.tile_pool(name="pv", bufs=2, space="PSUM") as pv_pool, \
             tc.tile_pool(name="aux", bufs=1, space="PSUM") as aux_pool, \
             tc.tile_pool(name="scr", bufs=1, space="PSUM") as scr_pool:
            pend_recip = []
            pend_norm = []
            pend_pe = []

            def proj_chunk(m, n, w_sb, x_sb, dst, bias_sb):
                def emit():
                    psq = scr_pool.tile([128, 512], F32, tag="scr",
                                        name=f"pj{m}{n}")
                    for c in range(8):
                        MM(psq[:],
                           w_sb[c][:, m * 128:(m + 1) * 128],
                           x_sb[c][:, n * 512:(n + 1) * 512],
                           start=(c == 0), stop=(c == 7))
                    nc.vector.tensor_scalar_add(
                        dst[m][:, n * 512:(n + 1) * 512], psq[:],
                        bias_sb[:, m:m + 1])
                return emit

            def k_chunk(m, n):
                return proj_chunk(m, n, wk_sb, xk_sb, kt, bk_sb)

            def q_chunk(m, n):
                return proj_chunk(m, n, wq_sb, xq_sb, qt, bq_sb)

            def out_piece(qb, fo, pool=None, tag="scr"):
                def emit():
                    yp = (pool or scr_pool).tile([128, 512], F32, tag=tag,
                                                 name=f"yp{qb}{fo}")
                    for pp in range(4):
                        MM(yp[:],
                           xT[pp][:, qb * 128:(qb + 1) * 128],
                           wo_sb[pp][:, fo * 512:(fo + 1) * 512],
                           start=(pp == 0), stop=(pp == 3))
                    ysb = y_sb_pool.tile([128, 512], F32, tag="ysb")
                    nc.vector.tensor_copy(ysb[:], yp[:])
                    eng = (nc.sync, nc.scalar, nc.gpsimd)[(2 * qb + fo) % 3]
                    eng.dma_start(
                        y[qb * 128:(qb + 1) * 128, fo * 512:(fo + 1) * 512],
                        ysb[:])
                return emit

            def flush_recips():
                for (fp, fqo, fzA, fzB) in pend_recip:
                    rzA = rz_pool.tile([1, 512], F32, tag="rzA", name="rzA")
                    rzB = rz_pool.tile([1, 512], F32, tag="rzB", name="rzB")
                    rzrA = rz_pool.tile([1, 512], F32R, tag="rzrA", name="rzrA")
                    rzrB = rz_pool.tile([1, 512], F32R, tag="rzrB", name="rzrB")
                    nc.vector.reciprocal_approx_fast(rzA[:], fzA[:])
                    nc.vector.reciprocal_approx_fast(rzB[:], fzB[:])
                    nc.vector.tensor_copy(rzrA[:], rzA[:])
                    nc.vector.tensor_copy(rzrB[:], rzB[:])
                    pend_norm.append((fp, fqo, rzrA, rzrB))
                pend_recip.clear()

            def flush_norms():
                for (fp, fqo, frzA, frzB) in pend_norm:
                    bc = pv_pool.tile([128, 512], F32, tag="pv", name="bc")
                    MM(bc[:], selA_sb[:], frzA[:], start=True, stop=False)
                    MM(bc[:], selB_sb[:], frzB[:], start=False, stop=True)
                    nc.vector.tensor_mul(xT[fp][:, fqo:fqo + 512],
                                         xT[fp][:, fqo:fqo + 512], bc[:])
                    if fp == 3:
                        qq = fqo // 512
                        pools = ([(None, "scr")] if qq < 3 else
                                 [(None, "scr"), (st_pool, "st"),
                                  (pv_pool, "pv"), (aux_pool, "zp")])
                        for i, (qb, fo) in enumerate(
                                (qb, fo) for qb in range(4 * qq, 4 * qq + 4)
                                for fo in range(2)):
                            pool, tag = pools[i % len(pools)]
                            push(12 + qq, out_piece(qb, fo, pool, tag))
                pend_norm.clear()

            def emit_pvz(pt, kb, p, pv, zp):
                # PV accumulation + Z row sums, two heads col-packed
                MM(pv[0:64, :],
                   v_sb[kb][:, p * 128:p * 128 + 64],
                   pt[:, 0:512],
                   tile_position=(0, 0),
                   start=(kb == 0), stop=(kb == 15))
                MM(pv[64:128, :],
                   v_sb[kb][:, p * 128 + 64:p * 128 + 128],
                   pt[:, 512:1024],
                   tile_position=(0, 64),
                   start=(kb == 0), stop=(kb == 15),
                   skip_group_check=True)
                MM(zp[0:64, :],
                   ones_sb[:],
                   pt[:, 0:512],
                   tile_position=(0, 0),
                   start=(kb == 0), stop=(kb == 15))
                MM(zp[64:128, :],
                   ones_sb[:],
                   pt[:, 512:1024],
                   tile_position=(0, 64),
                   start=(kb == 0), stop=(kb == 15),
                   skip_group_check=True)

            def finish_group(p, qq, pv, zp):
                # z extraction + x^T copy for a completed (p, qq) group
                qo = qq * 512
                zA = rz_pool.tile([1, 512], F32, tag="zA", name="zA", bufs=1)
                zB = rz_pool.tile([1, 512], F32, tag="zB", name="zB", bufs=1)
                nc.vector.tensor_copy(zA[:], zp[0:1, :])
                nc.vector.tensor_copy(zB[:], zp[64:65, :])
                nc.vector.tensor_copy(xT[p][:, qo:qo + 512], pv[:])
                pend_recip.append((p, qo, zA, zB))
                if p < 3:
                    push((p + 1) * 4 + qq - 0.4, q_chunk(p + 1, qq))

            import bisect

            def push(key, emit):
                bisect.insort(pend_pe, (key, next(_tie), emit))

            import itertools
            _tie = itertools.count()

            # head: first K and Q projection chunks only
            k_chunk(0, 0)()
            q_chunk(0, 0)()
            # remaining projection chunks, keyed by deadline (group index)
            for n in range(1, 4):
                push(n - 4, k_chunk(0, n))      # needed within group 0
            for n in range(1, 4):
                push(n, q_chunk(0, n))          # needed at group n
            for m in range(1, 4):
                for n in range(4):
                    push(4 * m - 0.5, k_chunk(m, n))  # needed at p=m

            pend_pvz = []        # FIFO of pending PV/Z emissions (depth 2)

            def drain_pvz(n_keep):
                while len(pend_pvz) > n_keep:
                    (pt_, kb_, p_, pv_, zp_, ginfo) = pend_pvz.pop(0)
                    emit_pvz(pt_, kb_, p_, pv_, zp_)
                    if ginfo is not None:
                        finish_group(*ginfo)

            for p in range(4):
                for qq in range(4):
                    qo = qq * 512
                    pv = pv_pool.tile([128, 512], F32, tag="pv")
                    zp = aux_pool.tile([128, 512], F32, tag="zp")
                    for kb in range(16):
                        if kb == 5 or (p == 3 and kb == 13):
                            flush_recips()
                        if kb == 9 or (p == 3 and kb == 1):
                            flush_norms()
                        ko = kb * 128
                        st = st_pool.tile([128, 1024], F32, tag="st")
                        # scores^T, two heads row-packed (K=64 each)
                        MM(st[:, 0:512],
                           kt[p][0:64, ko:ko + 128],
                           qt[p][0:64, qo:qo + 512],
                           start=True, stop=True)
                        MM(st[:, 512:1024],
                           kt[p][64:128, ko:ko + 128],
                           qt[p][64:128, qo:qo + 512],
                           start=True, stop=True)
                        pt = pt_pool.tile([128, 1024], BF16, tag="pt")
                        nc.scalar.activation(pt[:], st[:], EXP)
                        pend_pvz.append(
                            (pt, kb, p, pv, zp,
                             (p, qq, pv, zp) if kb == 15 else None))
                        drain_pvz(2)
                        if pend_pe:
                            gi = p * 4 + qq
                            urgent = pend_pe[0][0] <= gi + 1
                            if p == 3 or kb % 4 == 1 or (urgent and kb % 2 == 1):
                                pend_pe.pop(0)[2]()
            drain_pvz(0)
            flush_recips()
            flush_norms()
            for _, _, emit in pend_pe:
                emit()


def get_program():
    global _PROGRAM
    if _PROGRAM is None:
        _PROGRAM = _build_program()
    return _PROGRAM


def make_in_maps(Q_in, K_in, V_in, Wq, bq, Wk, bk, Wv, bv, Wo, bo):
    """Shard full inputs into 8 per-core input maps (bf16 pre-cast on host)."""
    scale = np.float32(1.0 / np.sqrt(DK))
    sel = np.zeros((2, 128), np.float32)
    sel[0, 0:64] = 1.0
    sel[1, 64:128] = 1.0
    ones = np.ones((128, 64), ml_dtypes.bfloat16)

    def b16(a):
        return np.ascontiguousarray(np.asarray(a, np.float32).astype(ml_dtypes.bfloat16))

    xt = {}
    for b in range(B):
        xt[b] = (b16(np.asarray(Q_in[b], np.float32).T),
                 b16(np.asarray(K_in[b], np.float32).T),
                 b16(np.asarray(V_in[b], np.float32).T))

    in_maps = []
    for c in range(N_CORES):
        b, hh = c // 2, c % 2
        sl = slice(hh * FEAT, (hh + 1) * FEAT)
        in_maps.append({
            "xq_t": xt[b][0],
            "xk_t": xt[b][1],
            "xv_t": xt[b][2],
            "wq": b16(np.asarray(Wq, np.float32)[:, sl]),
            "wk": b16(np.asarray(Wk, np.float32)[:, sl] * scale),
            "wv": b16(np.asarray(Wv, np.float32)[:, sl]),
            "wo": b16(np.asarray(Wo, np.float32)[sl, :]),
            "bq": np.ascontiguousarray(np.asarray(bq, np.float32)[sl, None]),
            "bk": np.ascontiguousarray(np.asarray(bk, np.float32)[sl, None] * scale),
            "sel": sel,
            "ones": ones,
        })
    return in_maps


def gather_output(results, Wo, bv, bo):
    """Combine per-core partial outputs into the full [B, S, D] output."""
    const = (np.asarray(bv, np.float32) @ np.asarray(Wo, np.float32)
             + np.asarray(bo, np.float32))
    out = np.empty((B, S, D), np.float32)
    for b in range(B):
        out[b] = results[2 * b]["y"] + results[2 * b + 1]["y"] + const
    return out


def kernel(Q_in, K_in, V_in, Wq, bq, Wk, bk, Wv, bv, Wo, bo):
    nc = get_program()
    in_maps = make_in_maps(Q_in, K_in, V_in, Wq, bq, Wk, bk, Wv, bv, Wo, bo)
    res = bass_utils.run_bass_kernel_spmd(nc, in_maps, core_ids=list(range(N_CORES)))
    return gather_output(res.results, Wo, bv, bo)


# revision 86
# speedup vs baseline: 1.0036x; 1.0036x over previous
"""Multi-head attention (B=4, S=2048, D=1024, H=16) on 8 Trainium2 cores.

Sharding: core c -> (batch b = c//2, head-half hh = c%2).  Each core computes
8 heads of one batch: QKV projections with column-sliced weights, attention,
and a partial output projection with row-sliced Wo.  Host sums the two
partial outputs per batch and adds the constant bias terms.

All matmul operands are bf16 (1 cyc/row streaming + FWL weight loads); host
pre-casts inputs/weights to bf16 so DMAs don't convert.  Everything is
computed in "transposed" orientation (features/keys on partitions):
  - Q^T, K^T: [512 feat, 2048 tok] via lhsT=W chunk, rhs=X^T chunk
  - V natural: [2048 tok, 512 feat] via lhsT=X^T chunk, rhs=Wv chunk
  - S^T[k,q]: lhsT=K^T[64,kblk], rhs=Q^T[64,qchunk] (two heads row-packed,
    concurrent in the PE via row groups)
  - P^T = exp(S^T) on ACT, PSUM->SBUF bf16 (no max-subtraction: |scores|<~6)
  - PV: out^T[dk,q] accum over kblk (two heads col-packed, concurrent); Z row
    sums via lhsT=ones alongside.  PV/Z for kblk i are emitted AFTER the
    scores matmul of kblk i+1 (software pipelining over the exp).
  - softmax normalization off the critical path: 1/Z via fast approx
    reciprocal on DVE, rounded to f32r, broadcast via rank-1 selector matmul,
    applied to x^T with a DVE mul
  - y[q, fout] natural: lhsT=x^T slice, rhs=Wo rows; per-128-row output
    blocks stream to HBM as soon as the last head block is normalized

The attention phase is ACT(exp)-bound (~1.15us per [128,1024] tile), so the
kernel minimizes the serial head before the first exp (V proj + K proj +
Q m=0 only) and interleaves the remaining Q projection chunks and the whole
output projection into the PE's slack inside the attention loop via a work
queue drained at kb boundaries.
"""
import numpy as np
import ml_dtypes

import concourse.tile as tile
from concourse import bacc, mybir
from concourse import bass_utils

F32 = mybir.dt.float32
F32R = mybir.dt.float32r
BF16 = mybir.dt.bfloat16
EXP = mybir.ActivationFunctionType.Exp

B, S, D = 4, 2048, 1024
H = 16
DK = 64
FEAT = 512          # features per core (8 heads)
N_CORES = 8

_PROGRAM = None


def _build_program():
    nc = bacc.Bacc("TRN2", target_bir_lowering=False, debug=False,
                   enable_asserts=True, num_devices=N_CORES)

    xq = nc.dram_tensor("xq_t", [D, S], BF16, kind="ExternalInput").ap()
    xk = nc.dram_tensor("xk_t", [D, S], BF16, kind="ExternalInput").ap()
    xv = nc.dram_tensor("xv_t", [D, S], BF16, kind="ExternalInput").ap()
    wq = nc.dram_tensor("wq", [D, FEAT], BF16, kind="ExternalInput").ap()
    wk = nc.dram_tensor("wk", [D, FEAT], BF16, kind="ExternalInput").ap()
    wv = nc.dram_tensor("wv", [D, FEAT], BF16, kind="ExternalInput").ap()
    wo = nc.dram_tensor("wo", [FEAT, D], BF16, kind="ExternalInput").ap()
    bq = nc.dram_tensor("bq", [FEAT, 1], F32, kind="ExternalInput").ap()
    bk = nc.dram_tensor("bk", [FEAT, 1], F32, kind="ExternalInput").ap()
    sel = nc.dram_tensor("sel", [2, 128], F32, kind="ExternalInput").ap()
    ones = nc.dram_tensor("ones", [128, 64], BF16, kind="ExternalInput").ap()
    y = nc.dram_tensor("y", [S, D], F32, kind="ExternalOutput").ap()

    with tile.TileContext(nc) as tc:
        with nc.allow_low_precision(reason="bf16/fp32r matmul operand tiles"):
            _emit(nc, tc, xq, xk, xv, wq, wk, wv, wo, bq, bk, sel, ones, y)
    nc.compile()
    return nc


def _emit(nc, tc, xq, xk, xv, wq, wk, wv, wo, bq, bk, sel, ones, y):
    from contextlib import ExitStack

    MM = nc.tensor.matmul

    with ExitStack() as ctx:
        ep = ctx.enter_context

        # ---------- persistent SBUF ----------
        qt_pool = ep(tc.tile_pool(name="qt", bufs=1))
        kt_pool = ep(tc.tile_pool(name="kt", bufs=1))
        v_pool = ep(tc.tile_pool(name="v", bufs=1))
        misc_pool = ep(tc.tile_pool(name="misc", bufs=1))
        xT_pool = ep(tc.tile_pool(name="xT", bufs=1))
        pt_pool = ep(tc.tile_pool(name="pt", bufs=3))
        rz_pool = ep(tc.tile_pool(name="rz", bufs=1))
        y_sb_pool = ep(tc.tile_pool(name="ysb", bufs=2))
        xk_pool = ep(tc.tile_pool(name="xk", bufs=1))
        wk_pool = ep(tc.tile_pool(name="wkp", bufs=1))
        xq_pool = ep(tc.tile_pool(name="xqp", bufs=1))
        wq_pool = ep(tc.tile_pool(name="wqp", bufs=1))

        qt = [qt_pool.tile([128, S], BF16, tag=f"qt{m}", name=f"qt{m}") for m in range(4)]
        kt = [kt_pool.tile([128, S], BF16, tag=f"kt{m}", name=f"kt{m}") for m in range(4)]
        v_sb = [v_pool.tile([128, FEAT], BF16, tag=f"v{k}", name=f"v{k}") for k in range(16)]
        xT = [xT_pool.tile([128, S], BF16, tag=f"xT{p}", name=f"xT{p}") for p in range(4)]

        bq_sb = misc_pool.tile([128, 4], F32, tag="bq")
        bk_sb = misc_pool.tile([128, 4], F32, tag="bk")
        ones_sb = misc_pool.tile([128, 64], BF16, tag="ones")
        selA_sb = misc_pool.tile([1, 128], F32R, tag="selA")
        selB_sb = misc_pool.tile([1, 128], F32R, tag="selB")


        xk_sb = [xk_pool.tile([128, S], BF16, tag=f"xk{c}", name=f"xk{c}") for c in range(8)]
        wk_sb = [wk_pool.tile([128, FEAT], BF16, tag=f"wk{c}", name=f"wk{c}") for c in range(8)]
        xq_sb = [xq_pool.tile([128, S], BF16, tag=f"xq{c}", name=f"xq{c}") for c in range(8)]
        wq_sb = [wq_pool.tile([128, FEAT], BF16, tag=f"wq{c}", name=f"wq{c}") for c in range(8)]

        # spread input DMAs across engine queues (one queue ~160GB/s):
        # V inputs split 3 ways so the V projection starts ASAP; K then Q
        # inputs follow on sync/scalar; weights trail on gpsimd/scalar.
        for m in range(4):
            nc.sync.dma_start(bq_sb[:, m:m + 1], bq[m * 128:(m + 1) * 128, 0:1])
            nc.sync.dma_start(bk_sb[:, m:m + 1], bk[m * 128:(m + 1) * 128, 0:1])
        nc.sync.dma_start(selA_sb[:], sel[0:1, :].bitcast(F32R))
        nc.sync.dma_start(selB_sb[:], sel[1:2, :].bitcast(F32R))


        # ---------- V projection (natural: [2048 tok, 512 feat]) ----------
        with tc.tile_pool(name="xv", bufs=1) as xv_pool, \
             tc.tile_pool(name="wvp", bufs=1) as wv_pool:
            xv_sb = [xv_pool.tile([128, S], BF16, tag=f"xv{c}", name=f"xv{c}")
                     for c in range(8)]
            wv_sb = [wv_pool.tile([128, FEAT], BF16, tag=f"wv{c}", name=f"wv{c}")
                     for c in range(8)]
            nc.gpsimd.dma_start(ones_sb[:], ones)
            for c in range(2):
                nc.gpsimd.dma_start(wv_sb[c][:], wv[c * 128:(c + 1) * 128, :])
                nc.gpsimd.dma_start(xv_sb[c][:], xv[c * 128:(c + 1) * 128, :])
            for c in range(2, 4):
                nc.scalar.dma_start(wv_sb[c][:], wv[c * 128:(c + 1) * 128, :])
                nc.scalar.dma_start(xv_sb[c][:], xv[c * 128:(c + 1) * 128, :])
            for c in range(4, 8):
                nc.sync.dma_start(wv_sb[c][:], wv[c * 128:(c + 1) * 128, :])
                nc.sync.dma_start(xv_sb[c][:], xv[c * 128:(c + 1) * 128, :])
            # K inputs on sync, Q inputs split sync/scalar, weights trail
            for c in range(8):
                nc.sync.dma_start(wk_sb[c][:], wk[c * 128:(c + 1) * 128, :])
                nc.sync.dma_start(xk_sb[c][:], xk[c * 128:(c + 1) * 128, :])
            for c in range(8):
                nc.gpsimd.dma_start(wq_sb[c][:], wq[c * 128:(c + 1) * 128, :])
            for c in range(4):
                nc.sync.dma_start(xq_sb[c][:], xq[c * 128:(c + 1) * 128, :])
            for c in range(4, 8):
                nc.scalar.dma_start(xq_sb[c][:], xq[c * 128:(c + 1) * 128, :])
            with tc.tile_pool(name="vps", bufs=1, space="PSUM") as vps_pool:
                for khalf in range(2):
                    vps = [vps_pool.tile([128, FEAT], F32, tag=f"vps{j}", name=f"vps{j}")
                           for j in range(8)]
                    for c in range(8):
                        for j in range(8):
                            kb = khalf * 8 + j
                            MM(vps[j][:],
                               xv_sb[c][:, kb * 128:(kb + 1) * 128],
                               wv_sb[c][:],
                               start=(c == 0), stop=(c == 7))
                    for j in range(8):
                        nc.vector.tensor_copy(v_sb[khalf * 8 + j][:], vps[j][:])

        # wo loads into the SBUF range freed by xv (needed only from ~p2 on)
        wo_pool = ep(tc.tile_pool(name="wo", bufs=1))
        wo_sb = [wo_pool.tile([128, D], BF16, tag=f"wo{p}", name=f"wo{p}") for p in range(4)]
        for p in range(4):
            nc.scalar.dma_start(wo_sb[p][:], wo[p * 128:(p + 1) * 128, :])

        # ---------- attention (ACT-bound) with interleaved PE work ----------
        # Pre-attention head is only K m=0 n=0 and Q m=0 n=0; every other
        # projection chunk plus the whole output projection is drained from a
        # work queue inside the ACT-paced attention loop.
        with tc.tile_pool(name="st", bufs=2, space="PSUM") as st_pool, \
             tc.tile_pool(name="pv", bufs=2, space="PSUM") as pv_pool, \
             tc.tile_pool(name="aux", bufs=1, space="PSUM") as aux_pool, \
             tc.tile_pool(name="scr", bufs=1, space="PSUM") as scr_pool:
            pend_recip = []
            pend_norm = []
            pend_pe = []

            def proj_chunk(m, n, w_sb, x_sb, dst, bias_sb):
                def emit():
                    psq = scr_pool.tile([128, 512], F32, tag="scr",
                                        name=f"pj{m}{n}")
                    for c in range(8):
                        MM(psq[:],
                           w_sb[c][:, m * 128:(m + 1) * 128],
                           x_sb[c][:, n * 512:(n + 1) * 512],
                           start=(c == 0), stop=(c == 7))
                    nc.vector.tensor_scalar_add(
                        dst[m][:, n * 512:(n + 1) * 512], psq[:],
                        bias_sb[:, m:m + 1])
                return emit

            def k_chunk(m, n):
                return proj_chunk(m, n, wk_sb, xk_sb, kt, bk_sb)

            def q_chunk(m, n):
                return proj_chunk(m, n, wq_sb, xq_sb, qt, bq_sb)

            def out_piece(qb, fo, pool=None, tag="scr"):
                def emit():
                    yp = (pool or scr_pool).tile([128, 512], F32, tag=tag,
                                                 name=f"yp{qb}{fo}")
                    for pp in range(4):
                        MM(yp[:],
                           xT[pp][:, qb * 128:(qb + 1) * 128],
                           wo_sb[pp][:, fo * 512:(fo + 1) * 512],
                           start=(pp == 0), stop=(pp == 3))
                    ysb = y_sb_pool.tile([128, 512], F32, tag="ysb")
                    nc.vector.tensor_copy(ysb[:], yp[:])
                    eng = (nc.sync, nc.scalar)[(2 * qb + fo) % 2]
                    eng.dma_start(
                        y[qb * 128:(qb + 1) * 128, fo * 512:(fo + 1) * 512],
                        ysb[:])
                return emit

            def flush_recips():
                for (fp, fqo, fzA, fzB) in pend_recip:
                    rzA = rz_pool.tile([1, 512], F32, tag="rzA", name="rzA")
                    rzB = rz_pool.tile([1, 512], F32, tag="rzB", name="rzB")
                    rzrA = rz_pool.tile([1, 512], F32R, tag="rzrA", name="rzrA")
                    rzrB = rz_pool.tile([1, 512], F32R, tag="rzrB", name="rzrB")
                    nc.vector.reciprocal_approx_fast(rzA[:], fzA[:])
                    nc.vector.reciprocal_approx_fast(rzB[:], fzB[:])
                    nc.vector.tensor_copy(rzrA[:], rzA[:])
                    nc.vector.tensor_copy(rzrB[:], rzB[:])
                    pend_norm.append((fp, fqo, rzrA, rzrB))
                pend_recip.clear()

            def flush_norms():
                for (fp, fqo, frzA, frzB) in pend_norm:
                    bc = pv_pool.tile([128, 512], F32, tag="pv", name="bc")
                    MM(bc[:], selA_sb[:], frzA[:], start=True, stop=False)
                    MM(bc[:], selB_sb[:], frzB[:], start=False, stop=True)
                    nc.vector.tensor_mul(xT[fp][:, fqo:fqo + 512],
                                         xT[fp][:, fqo:fqo + 512], bc[:])
                    if fp == 3:
                        qq = fqo // 512
                        pools = ([(None, "scr")] if qq < 3 else
                                 [(None, "scr"), (st_pool, "st"),
                                  (pv_pool, "pv"), (aux_pool, "zp")])
                        for i, (qb, fo) in enumerate(
                                (qb, fo) for qb in range(4 * qq, 4 * qq + 4)
                                for fo in range(2)):
                            pool, tag = pools[i % len(pools)]
                            push(12 + qq, out_piece(qb, fo, pool, tag))
                pend_norm.clear()

            def emit_pvz(pt, kb, p, pv, zp):
                # PV accumulation + Z row sums, two heads col-packed
                MM(pv[0:64, :],
                   v_sb[kb][:, p * 128:p * 128 + 64],
                   pt[:, 0:512],
                   tile_position=(0, 0),
                   start=(kb == 0), stop=(kb == 15))
                MM(pv[64:128, :],
                   v_sb[kb][:, p * 128 + 64:p * 128 + 128],
                   pt[:, 512:1024],
                   tile_position=(0, 64),
                   start=(kb == 0), stop=(kb == 15),
                   skip_group_check=True)
                MM(zp[0:64, :],
                   ones_sb[:],
                   pt[:, 0:512],
                   tile_position=(0, 0),
                   start=(kb == 0), stop=(kb == 15))
                MM(zp[64:128, :],
                   ones_sb[:],
                   pt[:, 512:1024],
                   tile_position=(0, 64),
                   start=(kb == 0), stop=(kb == 15),
                   skip_group_check=True)

            def finish_group(p, qq, pv, zp):
                # z extraction + x^T copy for a completed (p, qq) group
                qo = qq * 512
                zA = rz_pool.tile([1, 512], F32, tag="zA", name="zA", bufs=1)
                zB = rz_pool.tile([1, 512], F32, tag="zB", name="zB", bufs=1)
                nc.vector.tensor_copy(zA[:], zp[0:1, :])
                nc.vector.tensor_copy(zB[:], zp[64:65, :])
                nc.vector.tensor_copy(xT[p][:, qo:qo + 512], pv[:])
                pend_recip.append((p, qo, zA, zB))
                if p < 3:
                    push((p + 1) * 4 + qq - 0.4, q_chunk(p + 1, qq))

            import bisect

            def push(key, emit):
                bisect.insort(pend_pe, (key, next(_tie), emit))

            import itertools
            _tie = itertools.count()

            # head: first K and Q projection chunks only
            k_chunk(0, 0)()
            q_chunk(0, 0)()
            # remaining projection chunks, keyed by deadline (group index)
            for n in range(1, 4):
                push(n - 4, k_chunk(0, n))      # needed within group 0
            for n in range(1, 4):
                push(n, q_chunk(0, n))          # needed at group n
            for m in range(1, 4):
                for n in range(4):
                    push(4 * m - 0.5, k_chunk(m, n))  # needed at p=m

            pend_pvz = []        # FIFO of pending PV/Z emissions (depth 2)

            def drain_pvz(n_keep):
                while len(pend_pvz) > n_keep:
                    (pt_, kb_, p_, pv_, zp_, ginfo) = pend_pvz.pop(0)
                    emit_pvz(pt_, kb_, p_, pv_, zp_)
                    if ginfo is not None:
                        finish_group(*ginfo)

            for p in range(4):
                for qq in range(4):
                    qo = qq * 512
                    pv = pv_pool.tile([128, 512], F32, tag="pv")
                    zp = aux_pool.tile([128, 512], F32, tag="zp")
                    for kb in range(16):
                        if kb == 5 or (p == 3 and kb == 13):
                            flush_recips()
                        if kb == 9 or (p == 3 and kb == 1):
                            flush_norms()
                        ko = kb * 128
                        st = st_pool.tile([128, 1024], F32, tag="st")
                        # scores^T, two heads row-packed (K=64 each)
                        MM(st[:, 0:512],
                           kt[p][0:64, ko:ko + 128],
                           qt[p][0:64, qo:qo + 512],
                           start=True, stop=True)
                        MM(st[:, 512:1024],
                           kt[p][64:128, ko:ko + 128],
                           qt[p][64:128, qo:qo + 512],
                           start=True, stop=True)
                        pt = pt_pool.tile([128, 1024], BF16, tag="pt")
                        nc.scalar.activation(pt[:], st[:], EXP)
                        pend_pvz.append(
                            (pt, kb, p, pv, zp,
                             (p, qq, pv, zp) if kb == 15 else None))
                        drain_pvz(2)
                        if pend_pe:
                            gi = p * 4 + qq
                            urgent = pend_pe[0][0] <= gi + 1
                            if p == 3 or kb % 4 == 1 or (urgent and kb % 2 == 1):
                                pend_pe.pop(0)[2]()
            drain_pvz(0)
            flush_recips()
            flush_norms()
            for _, _, emit in pend_pe:
                emit()


def get_program():
    global _PROGRAM
    if _PROGRAM is None:
        _PROGRAM = _build_program()
    return _PROGRAM


def make_in_maps(Q_in, K_in, V_in, Wq, bq, Wk, bk, Wv, bv, Wo, bo):
    """Shard full inputs into 8 per-core input maps (bf16 pre-cast on host)."""
    scale = np.float32(1.0 / np.sqrt(DK))
    sel = np.zeros((2, 128), np.float32)
    sel[0, 0:64] = 1.0
    sel[1, 64:128] = 1.0
    ones = np.ones((128, 64), ml_dtypes.bfloat16)

    def b16(a):
        return np.ascontiguousarray(np.asarray(a, np.float32).astype(ml_dtypes.bfloat16))

    xt = {}
    for b in range(B):
        xt[b] = (b16(np.asarray(Q_in[b], np.float32).T),
                 b16(np.asarray(K_in[b], np.float32).T),
                 b16(np.asarray(V_in[b], np.float32).T))

    in_maps = []
    for c in range(N_CORES):
        b, hh = c // 2, c % 2
        sl = slice(hh * FEAT, (hh + 1) * FEAT)
        in_maps.append({
            "xq_t": xt[b][0],
            "xk_t": xt[b][1],
            "xv_t": xt[b][2],
            "wq": b16(np.asarray(Wq, np.float32)[:, sl]),
            "wk": b16(np.asarray(Wk, np.float32)[:, sl] * scale),
            "wv": b16(np.asarray(Wv, np.float32)[:, sl]),
            "wo": b16(np.asarray(Wo, np.float32)[sl, :]),
            "bq": np.ascontiguousarray(np.asarray(bq, np.float32)[sl, None]),
            "bk": np.ascontiguousarray(np.asarray(bk, np.float32)[sl, None] * scale),
            "sel": sel,
            "ones": ones,
        })
    return in_maps


def gather_output(results, Wo, bv, bo):
    """Combine per-core partial outputs into the full [B, S, D] output."""
    const = (np.asarray(bv, np.float32) @ np.asarray(Wo, np.float32)
             + np.asarray(bo, np.float32))
    out = np.empty((B, S, D), np.float32)
    for b in range(B):
        out[b] = results[2 * b]["y"] + results[2 * b + 1]["y"] + const
    return out


def kernel(Q_in, K_in, V_in, Wq, bq, Wk, bk, Wv, bv, Wo, bo):
    nc = get_program()
    in_maps = make_in_maps(Q_in, K_in, V_in, Wq, bq, Wk, bk, Wv, bv, Wo, bo)
    res = bass_utils.run_bass_kernel_spmd(nc, in_maps, core_ids=list(range(N_CORES)))
    return gather_output(res.results, Wo, bv, bo)
